# revision 1
# baseline (speedup 1.0000x reference)
"""Trainium2 Bass kernel for nn_EvoBinarizedLayer — fp8 DoubleRow + fp16 out, lean PE.

Math: out[p,b,o] = sum_i x[p,b,i]*w[0,p,i,o] + (1-x[p,b,i])*w[1,p,i,o]
                 = (x @ D)[p,b,o] + colsum(W1)[p,o],   D = W0 - W1

x is {0,1}, D is {-1,0,1}: both exact in fp8e4, so the matmuls run in
DoubleRow fp8 perf mode (2 contraction rows per PE cell, 2x throughput)
with exact fp32 PSUM accumulation.  colsum(W1) partials (<=8, integer)
are fp8-exact and enter each PSUM group as ones-matmuls.  Outputs are
integers in [0,1024]: exactly representable in fp16, so results are
stored as fp16 (halving store traffic) and upcast to f32 host-side with
zero error.

The kernel is HBM-streaming-bound (~46MB/core forced traffic):
 - w0 streams on the SP HWDGE ring, w1 on the ACT HWDGE ring (one ring
   alone caps well short of the HBM limit), 1MB chunks = one DoubleRow
   contraction step each.
 - x streams on the gpsimd SWDGE queue with f32->bf16 cast, is PE-
   transposed in bf16 and copied out as fp8.
 - stores stream per 128-row b-tile on gpsimd; the LAST population's
   tail is specially scheduled: its final 256 rows arrive as two
   128-row chunks whose bias colsums are pure copy-casts, the final
   copies split across ACT+DVE, and the final stores go to the
   (by then idle) HWDGE rings.

Sharding: population dim P=32 split across 8 cores (4 each), no
cross-core communication.
"""

import numpy as np

P, B, I, O = 32, 512, 1024, 1024
NCORES = 8
PPC = P // NCORES  # populations per core
NKP = I // 256     # DoubleRow k-pairs (256 contraction rows each)
NBT = B // 128     # b-tiles
NOH = O // 512     # o-halves (PSUM bank width)

_cache = {}

MAX_WAITS_PER_INST = 1


def _patch_tile_drain():
    """This container's walrus caps sem-waits per TPB_CTRL instruction below
    what Tile's final drain needs; spread the waits across nop instructions."""
    import concourse.tile as tile
    import bass_rust
    from concourse.vector_clock import ScopedClock

    if getattr(tile.TileContext, "_drain_patched", False):
        return

    def _drain_and_barrier(self, tick_clock, wait_clock):
        nc = self.nc
        drain_inst = nc.sync.drain()
        wait_clock.add_sem_waits(
            drain_inst.ins, ScopedClock({None: tick_clock.global_clock})
        )
        si = drain_inst.ins.sync_info
        waits = list(si.on_wait or [])
        if len(waits) > 1:
            si.on_wait = waits[:1]
            drain_inst.ins.sync_info = si
            for i in range(1, len(waits)):
                nop = nc.sync.nop()
                nop.ins.sync_info = bass_rust.SyncInfo(
                    on_wait=[waits[i]], on_update=[]
                )
        nc.all_engine_barrier()
        assert self.sems is not None
        popped = nc._tile_sem_poison_stack.pop()
        assert popped is self._sem_poison
        nc.clear_and_free_semaphores(list(self.sems.allocated().values()))
        nc.all_engine_barrier()

    tile.TileContext._drain_and_barrier = _drain_and_barrier
    tile.TileContext._drain_patched = True


def _split_excess_waits(nc):
    """This container's walrus rejects instructions carrying more than a
    couple of sem-waits; hoist excess waits onto same-engine nops placed
    just before the instruction."""
    import concourse.mybir as mybir
    import bass_rust

    n_split = 0
    for fn in nc.m.functions:
        for bb in fn.blocks:
            new_insts = []
            for inst in bb.instructions:
                si = inst.sync_info
                waits = list(si.on_wait) if si and si.on_wait else []
                if len(waits) > MAX_WAITS_PER_INST:
                    n_split += 1
                    extra = waits[: -MAX_WAITS_PER_INST]
                    keep = waits[-MAX_WAITS_PER_INST:]
                    for j in range(0, len(extra), MAX_WAITS_PER_INST):
                        nop = mybir.InstNoOp(
                            name=nc.get_next_instruction_name(), ins=[], outs=[]
                        )
                        nop.engine = inst.engine
                        nop.sync_info = bass_rust.SyncInfo(
                            on_wait=extra[j : j + MAX_WAITS_PER_INST], on_update=[]
                        )
                        nc.register_instruction(nop, overwrite=True)
                        new_insts.append(nop)
                    si.on_wait = keep
                    inst.sync_info = si
                new_insts.append(inst)
            bb.instructions = new_insts
    return n_split


def _build_nc():
    from contextlib import ExitStack

    import concourse.bass as bass
    import concourse.mybir as mybir
    import concourse.tile as tile
    from concourse.masks import make_identity

    _patch_tile_drain()

    f32 = mybir.dt.float32
    f16 = mybir.dt.float16
    bf16 = mybir.dt.bfloat16
    f8 = mybir.dt.float8e4
    DR = mybir.MatmulPerfMode.DoubleRow

    nc = bass.Bass()
    x_in = nc.declare_dram_parameter("x", [PPC, B, I], f32, isOutput=False)
    w_in = nc.declare_dram_parameter("w", [2, PPC, I, O], f32, isOutput=False)
    out_ext = nc.declare_dram_parameter("out", [PPC, B, O], f16, isOutput=True)

    with ExitStack() as ctx:
        tc = ctx.enter_context(tile.TileContext(nc))
        const_pool = ctx.enter_context(tc.tile_pool(name="const", bufs=1))
        w_pool = ctx.enter_context(tc.tile_pool(name="w", bufs=4))
        d_pool = ctx.enter_context(tc.tile_pool(name="d", bufs=2))
        s_pool = ctx.enter_context(tc.tile_pool(name="s", bufs=2))
        x_pool = ctx.enter_context(tc.tile_pool(name="xp", bufs=2))
        xt_pool = ctx.enter_context(tc.tile_pool(name="xt", bufs=2))
        out_pool = ctx.enter_context(tc.tile_pool(name="op", bufs=2))
        psum_mm = ctx.enter_context(tc.tile_pool(name="pmm", bufs=8, space="PSUM"))

        ident_bf = const_pool.tile([128, 128], bf16)
        make_identity(nc, ident_bf[:])
        ones_f8 = const_pool.tile([128, 128], f8)
        nc.gpsimd.memset(ones_f8[:], 1.0)

        for p in range(PPC):
            last = p == PPC - 1
            # ---- x: one 2MB SWDGE load with f32->bf16 cast
            x_p = x_pool.tile([128, NBT, I], bf16, name=f"x_{p}", tag="x")
            nc.gpsimd.dma_start(
                x_p[:], x_in[p].rearrange("(bt q) i -> q bt i", q=128)
            )
            # ---- weights: w0 on the SP ring, w1 on the ACT ring. 1MB
            #      chunks (one DoubleRow k-pair = 256 i-rows); the final
            #      k-pair arrives as two 128-row halves so the tail after
            #      the last byte is minimal.
            w_ts = []
            for j in range(NKP):
                w0_t = w_pool.tile([128, 2, O], f32, name=f"w0_{p}_{j}", tag="w0")
                w1_t = w_pool.tile([128, 2, O], f32, name=f"w1_{p}_{j}", tag="w1")
                sl = slice(j * 256, (j + 1) * 256)
                nc.sync.dma_start(
                    w0_t[:],
                    w_in[0, p, sl, :].rearrange("(a q) o -> q a o", q=128),
                )
                nc.scalar.dma_start(
                    w1_t[:],
                    w_in[1, p, sl, :].rearrange("(a q) o -> q a o", q=128),
                )
                w_ts.append((w0_t, w1_t))

            # ---- PE-transpose x (bf16) into fp8 xT tiles
            xT_p = xt_pool.tile([128, NBT, 2 * NKP, 128], f8, name=f"xT_{p}", tag="xT")
            for bt in range(NBT):
                ptr = psum_mm.tile(
                    [128, 2 * NKP, 128], bf16, name=f"ptr_{p}_{bt}", tag="g"
                )
                for c in range(2 * NKP):
                    nc.tensor.transpose(
                        ptr[:, c, :],
                        x_p[:, bt, c * 128 : (c + 1) * 128],
                        ident_bf[:],
                    )
                nc.scalar.copy(xT_p[:, bt], ptr[:])

            # ---- D = w0 - w1 (fp8) and bias colsum partials on DVE.
            # For p<last one merged bias matmul (full colsum tree); for the
            # last population the bias splits into ones@s012 (mid-stream)
            # + ones@sl4 (last chunk pair-colsum, the only tail DVE work),
            # with the last chunk's sub done per o-half.
            d_p = d_pool.tile([128, NKP, 2, O], f8, name=f"d_{p}", tag="d")
            s4 = [
                s_pool.tile([128, O], f32, name=f"s4_{p}_{j}", tag="s4", bufs=8)
                for j in range(NKP - 1)
            ]
            for j in range(NKP - 1):
                w0_t, w1_t = w_ts[j]
                nc.vector.tensor_sub(d_p[:, j], w0_t[:], w1_t[:])
                nc.vector.tensor_add(s4[j][:], w1_t[:, 0, :], w1_t[:, 1, :])
            s01 = s_pool.tile([128, O], f32, name=f"s01_{p}", tag="s01", bufs=1)
            nc.vector.tensor_add(s01[:], s4[0][:], s4[1][:])
            w0_l, w1_l = w_ts[NKP - 1]
            if not last:
                s012 = s_pool.tile([128, O], f32, name=f"s012_{p}", tag="s012")
                s_f8 = s_pool.tile([128, O], f8, name=f"sf8_{p}", tag="sf8")
                sl4 = s_pool.tile([128, O], f32, name=f"sl4_{p}", tag="sl4")
                nc.vector.tensor_add(s012[:], s01[:], s4[2][:])
                nc.vector.tensor_sub(d_p[:, NKP - 1], w0_l[:], w1_l[:])
                nc.vector.tensor_add(sl4[:], w1_l[:, 0, :], w1_l[:, 1, :])
                nc.vector.tensor_add(s_f8[:], s012[:], sl4[:])
            else:
                s012_f8 = s_pool.tile([128, O], f8, name=f"s012_{p}", tag="s012f8")
                sl4_f8 = s_pool.tile([128, O], f8, name=f"sl4f8_{p}", tag="sl4f8")
                nc.vector.tensor_add(s012_f8[:], s01[:], s4[2][:])
                for oh in range(NOH):
                    osl = slice(oh * 512, (oh + 1) * 512)
                    nc.vector.tensor_sub(
                        d_p[:, NKP - 1, :, osl], w0_l[:, :, osl], w1_l[:, :, osl]
                    )
                    nc.vector.tensor_add(
                        sl4_f8[:, osl], w1_l[:, 0, osl], w1_l[:, 1, osl]
                    )

            # ---- accumulation: 8 (bt, oh) PSUM groups
            groups = [(bt, oh) for bt in range(NBT) for oh in range(NOH)]
            pmms = {
                g: psum_mm.tile([128, 512], f32, name=f"pmm_{p}_{g[0]}_{g[1]}", tag="g")
                for g in groups
            }
            out_sb = out_pool.tile([128, NBT, O], f16, name=f"out_{p}", tag="out")
            for j in range(NKP - 1):
                for bt in range(NBT):
                    for oh in range(NOH):
                        nc.tensor.matmul(
                            pmms[(bt, oh)][:],
                            xT_p[:, bt, 2 * j : 2 * j + 2, :],
                            d_p[:, j, :, oh * 512 : (oh + 1) * 512],
                            start=(j == 0),
                            stop=False,
                            perf_mode=DR,
                        )
            j = NKP - 1
            if not last:
                # simple drain: last chunk + one merged bias matmul per
                # group, stores per b-tile on gpsimd
                for bt in range(NBT):
                    for oh in range(NOH):
                        nc.tensor.matmul(
                            pmms[(bt, oh)][:],
                            xT_p[:, bt, 2 * j : 2 * j + 2, :],
                            d_p[:, j, :, oh * 512 : (oh + 1) * 512],
                            start=False,
                            stop=False,
                            perf_mode=DR,
                        )
                    for oh in range(NOH):
                        osl = slice(oh * 512, (oh + 1) * 512)
                        nc.tensor.matmul(
                            pmms[(bt, oh)][:], ones_f8[:], s_f8[:, osl],
                            start=False, stop=True,
                        )
                        nc.scalar.copy(out_sb[:, bt, osl], pmms[(bt, oh)][:])
                    nc.gpsimd.dma_start(
                        out_ext[p, bt * 128 : (bt + 1) * 128, :], out_sb[:, bt]
                    )
            else:
                # tight tail: mid-stream bias part, then o-half-major last
                # chunk; copies split across ACT+DVE and stores per b-tile
                # on the (idle by now) HWDGE rings
                for bt in range(NBT):
                    for oh in range(NOH):
                        osl = slice(oh * 512, (oh + 1) * 512)
                        nc.tensor.matmul(
                            pmms[(bt, oh)][:], ones_f8[:], s012_f8[:, osl],
                            start=False, stop=False,
                        )
                for oh in range(NOH):
                    osl = slice(oh * 512, (oh + 1) * 512)
                    for bt in range(NBT):
                        nc.tensor.matmul(
                            pmms[(bt, oh)][:],
                            xT_p[:, bt, 2 * j : 2 * j + 2, :],
                            d_p[:, j, :, osl],
                            start=False,
                            stop=False,
                            perf_mode=DR,
                        )
                    for bt in range(NBT):
                        nc.tensor.matmul(
                            pmms[(bt, oh)][:], ones_f8[:], sl4_f8[:, osl],
                            start=False, stop=True,
                        )
                        if oh == 1:
                            if bt % 2 == 0:
                                nc.vector.tensor_copy(
                                    out_sb[:, bt, osl], pmms[(bt, oh)][:]
                                )
                            else:
                                nc.scalar.copy(out_sb[:, bt, osl], pmms[(bt, oh)][:])
                            st_eng = nc.sync if bt % 2 == 0 else nc.scalar
                            st_eng.dma_start(
                                out_ext[p, bt * 128 : (bt + 1) * 128, :],
                                out_sb[:, bt],
                            )
                        else:
                            nc.scalar.copy(out_sb[:, bt, osl], pmms[(bt, oh)][:])

    _split_excess_waits(nc)
    return nc


def get_nc():
    if "nc" not in _cache:
        _cache["nc"] = _build_nc()
    return _cache["nc"]


def run(x, w, trace=False, **kwargs):
    from concourse.bass_utils import run_bass_kernel_spmd

    x = np.ascontiguousarray(np.asarray(x, dtype=np.float32))
    w = np.ascontiguousarray(np.asarray(w, dtype=np.float32))
    assert x.shape == (P, B, I) and w.shape == (2, P, I, O)

    nc = get_nc()
    in_maps = [
        {
            "x": np.ascontiguousarray(x[c * PPC : (c + 1) * PPC]),
            "w": np.ascontiguousarray(w[:, c * PPC : (c + 1) * PPC]),
        }
        for c in range(NCORES)
    ]
    res = run_bass_kernel_spmd(nc, in_maps, list(range(NCORES)), trace=trace, **kwargs)
    # out is fp16 on-device (integers <= 1024, fp16-exact); upcast host-side
    out = np.concatenate([res.results[c]["out"] for c in range(NCORES)], axis=0)
    return out.astype(np.float32), res


def kernel(x, w):
    out, _ = run(x, w, trace=False)
    return out



# revision 2
# speedup vs baseline: 1.0615x; 1.0615x over previous
"""Trainium2 Bass kernel for nn_EvoBinarizedLayer — o-half streaming.

Math: out[p,b,o] = sum_i x[p,b,i]*w[0,p,i,o] + (1-x[p,b,i])*w[1,p,i,o]
                 = (x @ D)[p,b,o] + colsum(W1)[p,o],   D = W0 - W1

x is {0,1}, D is {-1,0,1}: both exact in fp8e4, so the matmuls run in
DoubleRow fp8 perf mode with exact fp32 PSUM accumulation.  Outputs are
integers <=1024, exact in fp16 (stored fp16, upcast host-side).

The kernel is HBM-streaming-bound (~44MB/core forced traffic at
~358 GB/s/core).  Design, in order of what bought time:

 - Weights stream in o-HALF-major order: for each population, all of
   w[:, :, 0:512] before w[:, :, 512:1024], in [256 i x 512 o] 0.5MB
   chunks (w0 on the SP HWDGE ring, w1 on ACT).  Each o-half is a
   complete accumulation problem over 4 PSUM groups (one per 128-row
   b-tile), so drains happen 8x per population and the final tail only
   drains the LAST o-half.  (1MB chunks measured SLOWER.)
 - w tile rings are 16 deep per tensor (2 populations of runway) so the
   HWDGE rings never starve on short engine stalls.
 - The SP/ACT instruction streams carry ONLY dma_starts mid-stream (no
   head-of-line blocking of DMA issue); for the LAST population both
   halves' weight DMAs issue before any drain work.  x loads (SWDGE
   bf16-cast) are prefetched 2-3 populations ahead; mid-stream stores
   go on SWDGE.
 - Bias colsum partials are stacked into a [128,4,512] fp8 tile and
   enter PSUM as ones-matmuls -- no serial DVE merge tree in the drain.
   All elementwise work stays on DVE (GpSimd tensor ops dilate DVE ~2x
   via SBUF contention -- measured).
 - PSUM: 2 banks reserved for x PE-transposes, 6 banks ring over the
   accumulation groups; transposes never collide with accumulation.
 - Tail: the final k-pair arrives as two 128-row halves (half-size last
   sub); tail PE work is FIFO-ordered by data readiness so only
   [mm j3, bias j3] remain after the last sub; copies split DVE/ACT and
   final stores go on the by-then-idle HWDGE rings.  ACT's activation
   table is pre-warmed so its first tail COPY doesn't pay ~1.3us.

Sharding: population dim P=32 split across 8 cores (4 each), no
cross-core communication.  Measured (quiet device): ~136.5us; the same
binary measures up to ~180us when neighbor tenants contend for HBM.
"""

import numpy as np

P, B, I, O = 32, 512, 1024, 1024
NCORES = 8
PPC = P // NCORES  # populations per core
NKP = I // 256     # DoubleRow k-pairs (256 contraction rows each)
NBT = B // 128     # b-tiles
NOH = O // 512     # o-halves (PSUM bank width)

_cache = {}

MAX_WAITS_PER_INST = 1


def _patch_tile_drain():
    """This container's walrus caps sem-waits per TPB_CTRL instruction below
    what Tile's final drain needs; spread the waits across nop instructions."""
    import concourse.tile as tile
    import bass_rust
    from concourse.vector_clock import ScopedClock

    if getattr(tile.TileContext, "_drain_patched", False):
        return

    def _drain_and_barrier(self, tick_clock, wait_clock):
        nc = self.nc
        drain_inst = nc.sync.drain()
        wait_clock.add_sem_waits(
            drain_inst.ins, ScopedClock({None: tick_clock.global_clock})
        )
        si = drain_inst.ins.sync_info
        waits = list(si.on_wait or [])
        if len(waits) > 1:
            si.on_wait = waits[:1]
            drain_inst.ins.sync_info = si
            for i in range(1, len(waits)):
                nop = nc.sync.nop()
                nop.ins.sync_info = bass_rust.SyncInfo(
                    on_wait=[waits[i]], on_update=[]
                )
        nc.all_engine_barrier()
        assert self.sems is not None
        popped = nc._tile_sem_poison_stack.pop()
        assert popped is self._sem_poison
        nc.clear_and_free_semaphores(list(self.sems.allocated().values()))
        nc.all_engine_barrier()

    tile.TileContext._drain_and_barrier = _drain_and_barrier
    tile.TileContext._drain_patched = True


def _split_excess_waits(nc):
    """This container's walrus rejects instructions carrying more than a
    couple of sem-waits; hoist excess waits onto same-engine nops placed
    just before the instruction."""
    import concourse.mybir as mybir
    import bass_rust

    n_split = 0
    for fn in nc.m.functions:
        for bb in fn.blocks:
            new_insts = []
            for inst in bb.instructions:
                si = inst.sync_info
                waits = list(si.on_wait) if si and si.on_wait else []
                if len(waits) > MAX_WAITS_PER_INST:
                    n_split += 1
                    extra = waits[: -MAX_WAITS_PER_INST]
                    keep = waits[-MAX_WAITS_PER_INST:]
                    for j in range(0, len(extra), MAX_WAITS_PER_INST):
                        nop = mybir.InstNoOp(
                            name=nc.get_next_instruction_name(), ins=[], outs=[]
                        )
                        nop.engine = inst.engine
                        nop.sync_info = bass_rust.SyncInfo(
                            on_wait=extra[j : j + MAX_WAITS_PER_INST], on_update=[]
                        )
                        nc.register_instruction(nop, overwrite=True)
                        new_insts.append(nop)
                    si.on_wait = keep
                    inst.sync_info = si
                new_insts.append(inst)
            bb.instructions = new_insts
    return n_split


def _build_nc():
    from contextlib import ExitStack

    import concourse.bass as bass
    import concourse.mybir as mybir
    import concourse.tile as tile
    from concourse.masks import make_identity

    _patch_tile_drain()

    f32 = mybir.dt.float32
    f16 = mybir.dt.float16
    bf16 = mybir.dt.bfloat16
    f8 = mybir.dt.float8e4
    DR = mybir.MatmulPerfMode.DoubleRow

    nc = bass.Bass()
    x_in = nc.declare_dram_parameter("x", [PPC, B, I], f32, isOutput=False)
    w_in = nc.declare_dram_parameter("w", [2, PPC, I, O], f32, isOutput=False)
    out_ext = nc.declare_dram_parameter("out", [PPC, B, O], f16, isOutput=True)

    with ExitStack() as ctx:
        tc = ctx.enter_context(tile.TileContext(nc))
        const_pool = ctx.enter_context(tc.tile_pool(name="const", bufs=1))
        w_pool = ctx.enter_context(tc.tile_pool(name="w", bufs=16))
        d_pool = ctx.enter_context(tc.tile_pool(name="d", bufs=8))
        s_pool = ctx.enter_context(tc.tile_pool(name="s", bufs=4))
        x_pool = ctx.enter_context(tc.tile_pool(name="xp", bufs=6))
        xt_pool = ctx.enter_context(tc.tile_pool(name="xt", bufs=2))
        o_pool = ctx.enter_context(tc.tile_pool(name="op", bufs=8))
        psum = ctx.enter_context(tc.tile_pool(name="ps", bufs=6, space="PSUM"))

        ident_bf = const_pool.tile([128, 128], bf16)
        make_identity(nc, ident_bf[:])
        ones_dr = const_pool.tile([128, 2, 128], f8)
        nc.gpsimd.memset(ones_dr[:], 1.0)
        # warm tile: ACT's first COPY triggers a ~1.3us ACT_TABLE_LOAD;
        # fire a throwaway copy early so it doesn't land in the tail drain
        warm_out = const_pool.tile([128, 16], f16)

        xtiles = {}

        def xload(p):
            for bh in range(2):
                t = x_pool.tile([128, 2, I], bf16, name=f"x_{p}_{bh}", tag="x")
                nc.gpsimd.dma_start(
                    t[:],
                    x_in[p, bh * 256 : (bh + 1) * 256, :].rearrange(
                        "(t q) i -> q t i", q=128
                    ),
                )
                xtiles[(p, bh)] = t

        # prefetch x for pops 0-2 up front; pop 3 is issued in pop 0's body
        xload(0)
        xload(1)
        xload(2)

        for p in range(PPC):
            last_p = p == PPC - 1
            if p + 3 < PPC:
                xload(p + 3)

            # ---- PE-transpose x (bf16) into fp8 xT tiles; 2 dedicated
            #      PSUM banks, copies out on DVE
            xT = xt_pool.tile([128, NBT, 2 * NKP, 128], f8, name=f"xT_{p}", tag="xT")
            for bt in range(NBT):
                ptr = psum.tile(
                    [128, 2 * NKP, 128], bf16, name=f"ptr_{p}_{bt}", tag="ptr", bufs=2
                )
                xsrc = xtiles[(p, bt // 2)]
                for c in range(2 * NKP):
                    nc.tensor.transpose(
                        ptr[:, c, :],
                        xsrc[:, bt % 2, c * 128 : (c + 1) * 128],
                        ident_bf[:],
                    )
                nc.vector.tensor_copy(xT[:, bt], ptr[:])

            # For the last population, issue BOTH halves' weight DMAs
            # before any drain work so neither HWDGE ring is ever
            # head-of-line blocked behind copies late in the stream.
            wts_by_h = {}
            for h in range(NOH):
                tail = last_p and h == NOH - 1
                osl = slice(h * 512, (h + 1) * 512)

                # ---- weight chunks for this o-half: w0 on SP, w1 on ACT.
                #      In the tail half the final k-pair arrives as two
                #      128-row halves so the last sub is half-size.
                wts = []
                for j in range(NKP):
                    w0_t = w_pool.tile(
                        [128, 2, 512], f32, name=f"w0_{p}_{h}_{j}", tag="w0"
                    )
                    w1_t = w_pool.tile(
                        [128, 2, 512], f32, name=f"w1_{p}_{h}_{j}", tag="w1"
                    )
                    if tail and j == NKP - 1:
                        for a in range(2):
                            sl = slice(j * 256 + a * 128, j * 256 + (a + 1) * 128)
                            nc.sync.dma_start(w0_t[:, a, :], w_in[0, p, sl, osl])
                            nc.scalar.dma_start(w1_t[:, a, :], w_in[1, p, sl, osl])
                    else:
                        sl = slice(j * 256, (j + 1) * 256)
                        nc.sync.dma_start(
                            w0_t[:],
                            w_in[0, p, sl, osl].rearrange("(a q) o -> q a o", q=128),
                        )
                        nc.scalar.dma_start(
                            w1_t[:],
                            w_in[1, p, sl, osl].rearrange("(a q) o -> q a o", q=128),
                        )
                    wts.append((w0_t, w1_t))
                if p == 0 and h == 0:
                    nc.scalar.copy(warm_out[:], ident_bf[:, 0:16])

                # ---- d = w0 - w1 (fp8) and stacked colsum partials (values
                #      <=2, fp8-exact), all on DVE.  (GpSimd adds dilate DVE
                #      ~2x via SBUF contention -- measured, don't.)
                s_stack = s_pool.tile([128, NKP, 512], f8, name=f"s_{p}_{h}", tag="s")
                dts = []
                for j in range(NKP):
                    w0_t, w1_t = wts[j]
                    d_t = d_pool.tile(
                        [128, 2, 512], f8, name=f"d_{p}_{h}_{j}", tag="d"
                    )
                    if tail and j == NKP - 1:
                        for a in range(2):
                            nc.vector.tensor_sub(
                                d_t[:, a, :], w0_t[:, a, :], w1_t[:, a, :]
                            )
                    else:
                        nc.vector.tensor_sub(d_t[:], w0_t[:], w1_t[:])
                    nc.vector.tensor_add(
                        s_stack[:, j, :], w1_t[:, 0, :], w1_t[:, 1, :]
                    )
                    dts.append(d_t)

                # ---- accumulation: 4 b-tile PSUM groups, chunk-major
                grps = [
                    psum.tile(
                        [128, 512], f32, name=f"g_{p}_{h}_{bt}", tag="grp", bufs=6
                    )
                    for bt in range(NBT)
                ]
                nj = NKP - 2 if tail else NKP
                for j in range(nj):
                    for bt in range(NBT):
                        nc.tensor.matmul(
                            grps[bt][:],
                            xT[:, bt, 2 * j : 2 * j + 2, :],
                            dts[j][:],
                            start=(j == 0),
                            stop=False,
                            perf_mode=DR,
                        )

                if not tail:
                    # ---- drain: two DR bias matmuls, copy, store per group
                    for bt in range(NBT):
                        nc.tensor.matmul(
                            grps[bt][:], ones_dr[:], s_stack[:, 0:2, :],
                            start=False, stop=False, perf_mode=DR,
                        )
                        nc.tensor.matmul(
                            grps[bt][:], ones_dr[:], s_stack[:, 2:4, :],
                            start=False, stop=True, perf_mode=DR,
                        )
                        ot = o_pool.tile(
                            [128, 512], f16, name=f"o_{p}_{h}_{bt}", tag="o"
                        )
                        if last_p:
                            # keep DVE clear for the tail half's subs
                            nc.scalar.copy(ot[:], grps[bt][:])
                        else:
                            nc.vector.tensor_copy(ot[:], grps[bt][:])
                        nc.gpsimd.dma_start(
                            out_ext[p, bt * 128 : (bt + 1) * 128, osl], ot[:]
                        )
                else:
                    # ---- tight tail, PE-FIFO ordered by data readiness:
                    #      [bias_a0, mm j2, bias_j2, mm j3, bias_j3] so the
                    #      only work after the final half-subs is
                    #      [mm j3, bias_j3] per group; copies split DVE/ACT,
                    #      stores on the (idle) HWDGE rings
                    ones_sq = ones_dr[:, 0, :]
                    for bt in range(NBT):
                        nc.tensor.matmul(
                            grps[bt][:], ones_dr[:], s_stack[:, 0:2, :],
                            start=False, stop=False, perf_mode=DR,
                        )
                    j = NKP - 2
                    for bt in range(NBT):
                        nc.tensor.matmul(
                            grps[bt][:],
                            xT[:, bt, 2 * j : 2 * j + 2, :],
                            dts[j][:],
                            start=False,
                            stop=False,
                            perf_mode=DR,
                        )
                    for bt in range(NBT):
                        nc.tensor.matmul(
                            grps[bt][:], ones_sq, s_stack[:, 2, :],
                            start=False, stop=False,
                        )
                    j = NKP - 1
                    for bt in range(NBT):
                        nc.tensor.matmul(
                            grps[bt][:],
                            xT[:, bt, 2 * j : 2 * j + 2, :],
                            dts[j][:],
                            start=False,
                            stop=False,
                            perf_mode=DR,
                        )
                    for bt in range(NBT):
                        nc.tensor.matmul(
                            grps[bt][:], ones_sq, s_stack[:, 3, :],
                            start=False, stop=True,
                        )
                        ot = o_pool.tile(
                            [128, 512], f16, name=f"o_{p}_{h}_{bt}", tag="o"
                        )
                        if bt % 2 == 0:
                            nc.vector.tensor_copy(ot[:], grps[bt][:])
                            nc.sync.dma_start(
                                out_ext[p, bt * 128 : (bt + 1) * 128, osl], ot[:]
                            )
                        else:
                            nc.scalar.copy(ot[:], grps[bt][:])
                            nc.scalar.dma_start(
                                out_ext[p, bt * 128 : (bt + 1) * 128, osl], ot[:]
                            )

    _split_excess_waits(nc)
    return nc


def get_nc():
    if "nc" not in _cache:
        _cache["nc"] = _build_nc()
    return _cache["nc"]


def run(x, w, trace=False, **kwargs):
    from concourse.bass_utils import run_bass_kernel_spmd

    x = np.ascontiguousarray(np.asarray(x, dtype=np.float32))
    w = np.ascontiguousarray(np.asarray(w, dtype=np.float32))
    assert x.shape == (P, B, I) and w.shape == (2, P, I, O)

    nc = get_nc()
    in_maps = [
        {
            "x": np.ascontiguousarray(x[c * PPC : (c + 1) * PPC]),
            "w": np.ascontiguousarray(w[:, c * PPC : (c + 1) * PPC]),
        }
        for c in range(NCORES)
    ]
    res = run_bass_kernel_spmd(nc, in_maps, list(range(NCORES)), trace=trace, **kwargs)
    # out is fp16 on-device (integers <= 1024, fp16-exact); upcast host-side
    out = np.concatenate([res.results[c]["out"] for c in range(NCORES)], axis=0)
    return out.astype(np.float32), res


def kernel(x, w):
    out, _ = run(x, w, trace=False)
    return out
